# revision 10
# baseline (speedup 1.0000x reference)
"""Llama4-style MoE (T=1024, H=2048, I=4096, E=8, top-1) on 8 trn2 NeuronCores.

Sharding: expert-parallel. Core e owns expert e's weights plus a 1/8 I-shard
of the shared expert. Host computes top-1 routing (tiny [1024,8] matmul) and
dispatches each expert's tokens (scaled by the sigmoid router score, padded to
capacity C) to its core. Each core returns its expert's MLP output plus a
partial shared-expert output; host sums the partials and scatters the routed
rows back.

Precision: routed expert weights are fp8 e3m4 (scaled x128, clipped +-15.5);
everything else bf16. The gate-path dequant folds into the SILU activation
(scale=1/128); the up/down path's 128^2 factor rides through bf16/fp32
(lossless power-of-2) and the host descales the routed output by 2^-14.
This halves the dominant HBM stream (routed weights 48->24 MB/core; total
~39 MB/core ~= 110 us at 358 GB/s), putting the kernel at the tensor-engine
bound: ~174 us of PE work at bf16 streaming speed (fp8 runs at the same
1 elem/cell/cycle without DoubleRow; DoubleRow's e4m3 mantissa is too coarse
for the 2e-2 gate).

DMA: the two HWDGE rings (sync/scalar) fair-share the ~358 GB/s HBM port at
packet granularity, so a ring carrying a stream that is needed NOW only gets
half bandwidth. All input streams are therefore split into <=0.5 MB subtiles
issued in consumption order and alternated ring-by-ring, which makes the two
FIFOs jointly approximate global earliest-deadline-first delivery. Outputs
ride SWDGE (gpsimd) or a late-idle HWDGE ring.

The PE clock boots throttled (~1.2 GHz) and needs ~3.4 us of sustained
activity: a memset-backed zero tile feeds warm-up matmuls from t~=0 with no
DMA dependency, so the ramp completes while the first real operands stream in.

Program order interleaves the DMA-dense routed groups with the resident-weight
shared-expert units so instantaneous input-byte demand stays under the HBM
rate: r0 sgu0 r1 sgu1 r2 sd0 r3 sgu2 r4 sgu3 r5 sd1 r6 r7 sd2 rd0 sd3 rd1-3.
"""

import numpy as np
import ml_dtypes

T, H, I, E = 1024, 2048, 4096, 8
P = 128
ISH = I // E          # 512  shared-expert I-shard per core
KH = H // P           # 16
MI = I // P           # 32
KSH = ISH // P        # 4
NT = 256              # shared-expert token chunk
NCH = T // NT         # 4 chunks

WS = 128.0            # fp8 weight scale (power of 2; exact to descale)
F8MAX = 15.5          # e3m4 max normal

BF = ml_dtypes.bfloat16
F8 = ml_dtypes.float8_e3m4

_BASS_CACHE = {}
LAST_RESULT = None    # BassKernelResults of the most recent run (for test harness)
LAST_NC = None


def _pack_runs(C):
    """Split the 4 m-tiles of a 512-wide group into runs that each fit one
    2 KB PSUM bank ([P, q, C] fp32 with q*C <= 512)."""
    pack = max(1, 512 // C)
    runs = []
    left = 4
    while left:
        q = min(pack, left)
        runs.append(q)
        left -= q
    return runs


def _build_bass(C):
    import concourse.bass as bass
    import concourse.mybir as mybir
    import concourse.tile as tile

    assert C <= 512, f"routed capacity {C} > 512 unsupported"

    f32 = mybir.dt.float32
    bf16 = mybir.dt.bfloat16
    f8 = mybir.dt.float8e3
    SILU = mybir.ActivationFunctionType.Silu
    MULT = mybir.AluOpType.mult

    nc = bass.Bass(trn_type="TRN2", name=f"moe_f8_c{C}")

    # --- DRAM tensors, all host-packed to [128, ...] partition-major ---
    xe3 = nc.dram_tensor("xe3", [P, KH, C], bf16, kind="ExternalInput")
    wgu = nc.dram_tensor("wgu", [P, 8, 2, KH, 512], f8, kind="ExternalInput")
    wd3 = nc.dram_tensor("wd3", [P, 4, MI, 512], f8, kind="ExternalInput")
    x3 = nc.dram_tensor("x3", [P, NCH, KH, NT], bf16, kind="ExternalInput")
    wsg3 = nc.dram_tensor("wsg3", [P, KH, ISH], bf16, kind="ExternalInput")
    wsu3 = nc.dram_tensor("wsu3", [P, KH, ISH], bf16, kind="ExternalInput")
    wsd3 = nc.dram_tensor("wsd3", [P, KSH, H], bf16, kind="ExternalInput")
    ro = nc.dram_tensor("ro", [P, KH, C], bf16, kind="ExternalOutput")
    sp = nc.dram_tensor("sp", [P, NCH, KH, NT], bf16, kind="ExternalOutput")

    runs = _pack_runs(C)

    with tile.TileContext(nc) as tc:
        from contextlib import ExitStack

        with ExitStack() as ctx:
            const = ctx.enter_context(tc.tile_pool(name="const", bufs=1))
            xpool = ctx.enter_context(tc.tile_pool(name="xpool", bufs=2))
            wpool = ctx.enter_context(tc.tile_pool(name="wpool", bufs=10))
            wdpool = ctx.enter_context(tc.tile_pool(name="wdpool", bufs=8))
            hsp = ctx.enter_context(tc.tile_pool(name="hsp", bufs=2))
            hbuf = ctx.enter_context(tc.tile_pool(name="hbuf", bufs=2))
            outp = ctx.enter_context(tc.tile_pool(name="outp", bufs=2))
            psum = ctx.enter_context(tc.tile_pool(name="psum", bufs=8, space="PSUM"))
            hTp = ctx.enter_context(tc.tile_pool(name="hTp", bufs=1))

            # ring alternator for input streams: both HWDGE FIFOs carry every
            # stream in consumption order, halving the skew a dedicated-ring
            # split would produce under 50/50 packet arbitration.
            rings = [nc.sync, nc.scalar]
            rstate = [0]

            def ring():
                r = rings[rstate[0] & 1]
                rstate[0] += 1
                return r

            # --- PE warm-up from a memset tile: no DMA dependency, so the
            # clock ramp (~3.4 us at boot) overlaps the DMA cold start, which
            # delivers only ~0.15 MB/us until ~26 us.
            wz = const.tile([P, 512], bf16, name="wz")
            nc.gpsimd.memset(wz, 0.0)
            warm = psum.tile([P, 512], f32, tag="ps", name="warm")
            for _ in range(14):
                nc.tensor.matmul(warm, wz[:, 0:P], wz[:, :],
                                 start=True, stop=True)

            # --- input loads, subtiled <=0.5 MB, in consumption order ---
            xeT = const.tile([P, KH, C], bf16)

            xts = [None] * NCH

            def load_x(t, nq=2):
                # k-quarters/halves: chunk matmuls unblock per k-slice
                xts[t] = xpool.tile([P, KH, NT], bf16, tag="xt", name=f"xt{t}")
                kq = KH // nq
                for hh in range(nq):
                    sl = slice(hh * kq, (hh + 1) * kq)
                    ring().dma_start(out=xts[t][:, sl, :],
                                     in_=x3.ap()[:, t, sl, :])

            wsg_sb = const.tile([P, KH, ISH], bf16)
            wsu_sb = const.tile([P, KH, ISH], bf16)
            wsd_sb = const.tile([P, KSH, H], bf16)

            def load_shared_w(wsb, src, nq=4):
                nk = wsb.shape[1]
                kq = nk // nq
                for hh in range(nq):
                    sl = slice(hh * kq, (hh + 1) * kq)
                    ring().dma_start(out=wsb[:, sl, :], in_=src.ap()[:, sl, :])

            hT = hTp.tile([P, MI, C], bf16)
            ro_sb = const.tile([P, KH, C], bf16, name="ro_sb")

            KHH = KH // 2  # half-group k tiles

            def routed_gu_group(g):
                """Routed expert gate/up for one 512-wide I group -> hT.

                fp8 weights stream in 0.5 MB half-tiles. Group 0 (the cold-DMA
                window) gets 0.25 MB gate quarters hand-placed on the rings,
                interleaved with the xeT halves, so both FIFOs deliver the
                first matmuls' operands concurrently."""
                nq = 4 if g == 0 else 2
                qt = {}

                def wtile(w, hh, kq):
                    wB = wpool.tile([P, kq, 512], f8, tag="w",
                                    name=f"w{g}_{w}_{hh}",
                                    padded_shape=[P, KHH, 512])
                    qt[(w, hh)] = (wB, kq)
                    return wB, wgu.ap()[:, g, w, hh * kq:(hh + 1) * kq, :]

                if g == 0:
                    KHQ = KH // 4
                    gq = [wtile(0, hh, KHQ) for hh in range(4)]
                    uh = [wtile(1, hh, KHH) for hh in range(2)]
                    xe_sl = [slice(0, KHH), slice(KHH, KH)]
                    nc.sync.dma_start(out=xeT[:, xe_sl[0], :],
                                      in_=xe3.ap()[:, xe_sl[0], :])
                    nc.scalar.dma_start(out=gq[0][0], in_=gq[0][1])
                    nc.sync.dma_start(out=gq[1][0], in_=gq[1][1])
                    nc.scalar.dma_start(out=xeT[:, xe_sl[1], :],
                                        in_=xe3.ap()[:, xe_sl[1], :])
                    nc.scalar.dma_start(out=gq[2][0], in_=gq[2][1])
                    nc.sync.dma_start(out=gq[3][0], in_=gq[3][1])
                    nc.sync.dma_start(out=uh[0][0], in_=uh[0][1])
                    nc.scalar.dma_start(out=uh[1][0], in_=uh[1][1])
                else:
                    for w in range(2):
                        for hh in range(2):
                            wB, src = wtile(w, hh, KHH)
                            ring().dma_start(out=wB, in_=src)
                gps, ups, mmap = [], [], []
                for ri, q in enumerate(runs):
                    gps.append(psum.tile([P, q, C], f32, tag="ps",
                                         name=f"gps{g}_{ri}"))
                    ups.append(psum.tile([P, q, C], f32, tag="ps",
                                         name=f"ups{g}_{ri}"))
                    for j in range(q):
                        mmap.append((ri, j))
                for w, ps_tiles in ((0, gps), (1, ups)):
                    for k in range(KH):
                        nw = nq if w == 0 else 2
                        kq = KH // nw
                        wB, _ = qt[(w, k // kq)]
                        for mi in range(4):
                            ri, j = mmap[mi]
                            # start only on the first write to each PSUM bank:
                            # start=True clears the whole bank's has_written.
                            st = dict(start=(k == 0 and j == 0),
                                      stop=(k == KH - 1))
                            nc.tensor.matmul(ps_tiles[ri][:, j, :],
                                             wB[:, k % kq, mi * P:(mi + 1) * P],
                                             xeT[:, k, :], **st)
                off = 0
                for ri, q in enumerate(runs):
                    h_sb = hbuf.tile([P, q, C], bf16, tag="hrb",
                                     name=f"hrb{g}_{ri}")
                    # dequant: PSUM holds 128*gate; silu(in/128)
                    nc.scalar.activation(out=h_sb, in_=gps[ri], func=SILU,
                                         scale=1.0 / WS)
                    # h = silu(gate) * (128*up): hT carries a 128x scale
                    nc.vector.tensor_tensor(hT[:, g * 4 + off:g * 4 + off + q, :],
                                            h_sb, ups[ri], MULT)
                    off += q

            hss = [None] * NCH

            def shared_gu(t):
                """Shared expert gate/up for one 256-token chunk -> hs[t]."""
                xt = xts[t]
                # pack two [P, NT] fp32 accumulators per PSUM bank
                sg = [psum.tile([P, 2, NT], f32, tag="ps", name=f"sg{t}_{r}")
                      for r in range(2)]
                su = [psum.tile([P, 2, NT], f32, tag="ps", name=f"su{t}_{r}")
                      for r in range(2)]
                for ps_tiles, wB in ((sg, wsg_sb), (su, wsu_sb)):
                    for k in range(KH):
                        for m in range(KSH):
                            st = dict(start=(k == 0 and m % 2 == 0),
                                      stop=(k == KH - 1))
                            nc.tensor.matmul(ps_tiles[m // 2][:, m % 2, :],
                                             wB[:, k, m * P:(m + 1) * P],
                                             xt[:, k, :], **st)
                hs = hsp.tile([P, KSH, NT], bf16, tag="hs", name=f"hs{t}")
                hss[t] = hs
                for r in range(2):
                    htmp = hbuf.tile([P, 2, NT], bf16, tag="hsb",
                                     name=f"htmp{t}_{r}")
                    nc.scalar.activation(out=htmp, in_=sg[r], func=SILU)
                    nc.vector.tensor_tensor(hs[:, 2 * r:2 * r + 2, :], htmp,
                                            su[r], MULT)

            def shared_down(t, final=False):
                """Shared expert down-proj for chunk t -> sp.

                The final chunk runs last in the kernel: its sp output leaves
                in 0.25 MB subtiles (2 KB DRAM lines) as each 4-m2 block's
                copies land, so the kernel tail is one short transfer."""
                hs = hss[t]
                sp_sb = outp.tile([P, KH, NT], bf16, tag="spsb", name=f"spsb{t}",
                                  bufs=1)
                for m2 in range(KH):
                    ps = psum.tile([P, NT], f32, tag="ps", name=f"sps{t}_{m2}")
                    for k2 in range(KSH):
                        nc.tensor.matmul(ps, wsd_sb[:, k2, m2 * P:(m2 + 1) * P],
                                         hs[:, k2, :],
                                         start=(k2 == 0), stop=(k2 == KSH - 1))
                    nc.vector.tensor_copy(out=sp_sb[:, m2, :], in_=ps)
                    if final and m2 % 4 == 3:
                        sl = slice(m2 - 3, m2 + 1)
                        nc.scalar.dma_start(out=sp.ap()[:, t, sl, :],
                                            in_=sp_sb[:, sl, :])
                if not final:
                    nc.gpsimd.dma_start(out=sp.ap()[:, t], in_=sp_sb)

            # wd streams as 16 x 0.5 MB fp8 chunks (8 k2-tiles each); the
            # first six are hoisted into the gate/up phase, the rest queue at
            # the start of the down phase and fire as ring slots free.
            wd_tiles = {}

            def prefetch_wd(c):
                if c in wd_tiles or c >= 16:
                    return
                wdB = wdpool.tile([P, KHH, 512], f8, tag="wd",
                                  name=f"wdB{c}")
                g2, q = c // 4, c % 4
                ring().dma_start(
                    out=wdB, in_=wd3.ap()[:, g2, q * KHH:(q + 1) * KHH, :])
                wd_tiles[c] = wdB

            def routed_down_group(g2):
                """Routed expert down-proj for one 512-wide H group -> ro_sb."""
                for c in range(6, 16):
                    prefetch_wd(c)
                dps, mmap = [], []
                for ri, q in enumerate(runs):
                    dps.append(psum.tile([P, q, C], f32, tag="ps",
                                         name=f"dps{g2}_{ri}"))
                    for j in range(q):
                        mmap.append((ri, j))
                for k2 in range(MI):
                    wb = wd_tiles[g2 * 4 + k2 // KHH]
                    kk = k2 % KHH
                    for mi in range(4):
                        ri, j = mmap[mi]
                        st = dict(start=(k2 == 0 and j == 0),
                                  stop=(k2 == MI - 1))
                        nc.tensor.matmul(dps[ri][:, j, :],
                                         wb[:, kk, mi * P:(mi + 1) * P],
                                         hT[:, k2, :], **st)
                # ro carries a 128^2 scale (fp8 wd x fp8-gated h); host
                # descales by 2^-14 exactly.
                off = 0
                for ri, q in enumerate(runs):
                    nc.vector.tensor_copy(out=ro_sb[:, g2 * 4 + off:
                                                    g2 * 4 + off + q, :],
                                          in_=dps[ri])
                    off += q
                # ro leaves in 8-m-tile halves: 2304 B DRAM lines (one
                # 288 B line per m-tile would run far below line rate)
                if g2 == 1:
                    nc.gpsimd.dma_start(out=ro.ap()[:, 0:8, :],
                                        in_=ro_sb[:, 0:8, :])
                elif g2 == 3:
                    nc.sync.dma_start(out=ro.ap()[:, 8:16, :],
                                      in_=ro_sb[:, 8:16, :])

            # Issue order == consumption order (the ring alternator assumes
            # it). Shared units are placed so their input-byte needs fit the
            # HBM rate alongside the routed weight stream.
            routed_gu_group(0)
            load_x(0, nq=4)
            load_shared_w(wsg_sb, wsg3, nq=4)
            load_shared_w(wsu_sb, wsu3, nq=4)
            shared_gu(0)
            routed_gu_group(1)
            load_x(1)
            shared_gu(1)
            routed_gu_group(2)
            load_shared_w(wsd_sb, wsd3, nq=2)
            shared_down(0)
            routed_gu_group(3)
            load_x(2)
            shared_gu(2)
            routed_gu_group(4)
            load_x(3)
            shared_gu(3)
            routed_gu_group(5)
            shared_down(1)
            for c in range(4):
                prefetch_wd(c)
            routed_gu_group(6)
            for c in range(4, 6):
                prefetch_wd(c)
            routed_gu_group(7)
            shared_down(2)
            routed_down_group(0)
            routed_down_group(1)
            routed_down_group(2)
            routed_down_group(3)
            # the shared chunk-3 down-proj closes the kernel: resident
            # weights, fat-line subtile outputs -> shortest possible tail
            shared_down(3, final=True)

    # Split surplus semaphore waits onto InstEventSemaphore carriers (walrus
    # has a 1-wait limit per instruction).
    import bass_rust
    bass_rust.generate_event_semaphores(nc)
    return nc


def _get_bass(C):
    if C not in _BASS_CACHE:
        _BASS_CACHE[C] = _build_bass(C)
    return _BASS_CACHE[C]


def _q8(w):
    """fp32 weight block -> e3m4 scaled by WS, clipped to the format max."""
    return np.clip(w * WS, -F8MAX, F8MAX).astype(F8)


def kernel(**inputs):
    global LAST_RESULT, LAST_NC
    x = np.ascontiguousarray(np.asarray(inputs["x"], dtype=np.float32))
    w_router = np.asarray(inputs["w_router"], dtype=np.float32)
    ws_gate = np.asarray(inputs["ws_gate"], dtype=np.float32)
    ws_up = np.asarray(inputs["ws_up"], dtype=np.float32)
    ws_down = np.asarray(inputs["ws_down"], dtype=np.float32)
    we_gate = np.asarray(inputs["we_gate"], dtype=np.float32)
    we_up = np.asarray(inputs["we_up"], dtype=np.float32)
    we_down = np.asarray(inputs["we_down"], dtype=np.float32)

    # --- top-1 routing on host (tiny) ---
    logits = x @ w_router                      # [T, E]
    top = np.argmax(logits, axis=1)            # [T]
    tv = logits[np.arange(T), top]
    score = (1.0 / (1.0 + np.exp(-tv))).astype(np.float32)
    all_idxs = [np.nonzero(top == e)[0] for e in range(E)]
    maxn = max(len(i) for i in all_idxs)
    C = min(512, max(P, ((maxn + 15) // 16) * 16))
    # capacity overflow (not hit for balanced routing): process the routed
    # tokens in multiple passes of <= 512; only pass 0's shared output counts.
    n_pass = (maxn + C - 1) // C

    nc = _get_bass(C)
    LAST_NC = nc

    # x3[p, t, k, j] = x[t*NT + j, k*128 + p]
    x3 = x.reshape(NCH, NT, KH, P).transpose(3, 0, 2, 1).astype(BF)

    from concourse.bass_utils import run_bass_kernel_spmd

    out = None
    for p_i in range(n_pass):
        idxs = [i[p_i * C:(p_i + 1) * C] for i in all_idxs]
        in_maps = []
        for e in range(E):
            idx = idxs[e]
            xe = np.zeros((C, H), np.float32)
            if len(idx):
                xe[:len(idx)] = x[idx] * score[idx, None]
            # xe3[p, k, c] = xe[c, k*128 + p]
            xe3 = xe.reshape(C, KH, P).transpose(2, 1, 0).astype(BF)

            # wgu[p, g, w, k, j] = we_{gate,up}[e][k*128 + p, g*512 + j]
            wgu = np.empty((P, 8, 2, KH, 512), F8)
            wgu[:, :, 0] = _q8(we_gate[e]).reshape(KH, P, 8, 512) \
                .transpose(1, 2, 0, 3)
            wgu[:, :, 1] = _q8(we_up[e]).reshape(KH, P, 8, 512) \
                .transpose(1, 2, 0, 3)
            # wd3[p, g2, k2, j] = we_down[e][k2*128 + p, g2*512 + j]
            wd3 = _q8(we_down[e]).reshape(MI, P, 4, 512) \
                .transpose(1, 2, 0, 3)

            # shared-expert shard for this core (bf16)
            wsg3 = ws_gate[:, e * ISH:(e + 1) * ISH].reshape(KH, P, ISH) \
                .transpose(1, 0, 2).astype(BF)
            wsu3 = ws_up[:, e * ISH:(e + 1) * ISH].reshape(KH, P, ISH) \
                .transpose(1, 0, 2).astype(BF)
            wsd3 = ws_down[e * ISH:(e + 1) * ISH].reshape(KSH, P, H) \
                .transpose(1, 0, 2).astype(BF)

            in_maps.append({
                "xe3": xe3, "wgu": wgu, "wd3": wd3, "x3": x3,
                "wsg3": wsg3, "wsu3": wsu3, "wsd3": wsd3,
            })

        res = run_bass_kernel_spmd(nc, in_maps, core_ids=list(range(E)))
        LAST_RESULT = res
        outs = res.results

        if p_i == 0:
            # shared partials: sp[p, t, m2, j] -> [token, h], sum over cores
            spsum = np.zeros((P, NCH, KH, NT), np.float32)
            for e in range(E):
                spsum += outs[e]["sp"].astype(np.float32)
            out = np.ascontiguousarray(
                spsum.transpose(1, 3, 2, 0).reshape(T, H))

        # routed: ro[p, m, c] -> [c, h], scatter back by token index,
        # descaling the fp8 weight scale (WS^2) exactly.
        dsc = np.float32(1.0 / (WS * WS))
        for e in range(E):
            idx = idxs[e]
            if len(idx):
                roe = outs[e]["ro"].astype(np.float32) * dsc
                out[idx] += roe.transpose(2, 1, 0).reshape(C, H)[:len(idx)]
    return out


# revision 14
# speedup vs baseline: 1.0038x; 1.0038x over previous
"""Llama4-style MoE (T=1024, H=2048, I=4096, E=8, top-1) on 8 trn2 NeuronCores.

Sharding: expert-parallel. Core e owns expert e's weights plus a 1/8 I-shard
of the shared expert. Host computes top-1 routing (tiny [1024,8] matmul) and
dispatches each expert's tokens (scaled by the sigmoid router score, padded to
capacity C) to its core. Each core returns its expert's MLP output plus a
partial shared-expert output; host sums the partials and scatters the routed
rows back.

Precision: routed expert weights are fp8 e3m4 (scaled x128, clipped +-15.5);
everything else bf16. The gate-path dequant folds into the SILU activation
(scale=1/128); the up/down path's 128^2 factor rides through bf16/fp32
(lossless power-of-2) and the host descales the routed output by 2^-14.
This halves the dominant HBM stream (routed weights 48->24 MB/core; total
~39 MB/core ~= 110 us at 358 GB/s), putting the kernel at the tensor-engine
bound: ~174 us of PE work at bf16 streaming speed (fp8 runs at the same
1 elem/cell/cycle without DoubleRow; DoubleRow's e4m3 mantissa is too coarse
for the 2e-2 gate).

DMA: the two HWDGE rings (sync/scalar) fair-share the ~358 GB/s HBM port at
packet granularity, so a ring carrying a stream that is needed NOW only gets
half bandwidth. All input streams are therefore split into <=0.5 MB subtiles
issued in consumption order and alternated ring-by-ring, which makes the two
FIFOs jointly approximate global earliest-deadline-first delivery. Outputs
ride SWDGE (gpsimd) or a late-idle HWDGE ring.

The PE clock boots throttled (~1.2 GHz) and needs ~3.4 us of sustained
activity: a memset-backed zero tile feeds warm-up matmuls from t~=0 with no
DMA dependency, so the ramp completes while the first real operands stream in.

Program order interleaves the DMA-dense routed groups with the resident-weight
shared-expert units so instantaneous input-byte demand stays under the HBM
rate: r0 sgu0 r1 sgu1 r2 sd0 r3 sgu2 r4 sgu3 r5 sd1 r6 r7 sd2 rd0 sd3 rd1-3.
"""

import numpy as np
import ml_dtypes

T, H, I, E = 1024, 2048, 4096, 8
P = 128
ISH = I // E          # 512  shared-expert I-shard per core
KH = H // P           # 16
MI = I // P           # 32
KSH = ISH // P        # 4
NT = 256              # shared-expert token chunk
NCH = T // NT         # 4 chunks

WS = 128.0            # fp8 weight scale (power of 2; exact to descale)
F8MAX = 15.5          # e3m4 max normal

BF = ml_dtypes.bfloat16
F8 = ml_dtypes.float8_e3m4

_BASS_CACHE = {}
LAST_RESULT = None    # BassKernelResults of the most recent run (for test harness)
LAST_NC = None


def _pack_runs(C):
    """Split the 4 m-tiles of a 512-wide group into runs that each fit one
    2 KB PSUM bank ([P, q, C] fp32 with q*C <= 512)."""
    pack = max(1, 512 // C)
    runs = []
    left = 4
    while left:
        q = min(pack, left)
        runs.append(q)
        left -= q
    return runs


def _build_bass(C):
    import concourse.bass as bass
    import concourse.mybir as mybir
    import concourse.tile as tile

    assert C <= 512, f"routed capacity {C} > 512 unsupported"

    f32 = mybir.dt.float32
    bf16 = mybir.dt.bfloat16
    f8 = mybir.dt.float8e3
    SILU = mybir.ActivationFunctionType.Silu
    MULT = mybir.AluOpType.mult

    nc = bass.Bass(trn_type="TRN2", name=f"moe_f8_c{C}")

    # --- DRAM tensors, all host-packed to [128, ...] partition-major ---
    xe3 = nc.dram_tensor("xe3", [P, KH, C], bf16, kind="ExternalInput")
    wgu = nc.dram_tensor("wgu", [P, 8, 2, KH, 512], f8, kind="ExternalInput")
    wd3 = nc.dram_tensor("wd3", [P, 4, MI, 512], f8, kind="ExternalInput")
    x3 = nc.dram_tensor("x3", [P, NCH, KH, NT], bf16, kind="ExternalInput")
    wsg3 = nc.dram_tensor("wsg3", [P, KH, ISH], bf16, kind="ExternalInput")
    wsu3 = nc.dram_tensor("wsu3", [P, KH, ISH], bf16, kind="ExternalInput")
    wsd3 = nc.dram_tensor("wsd3", [P, KSH, H], bf16, kind="ExternalInput")
    ro = nc.dram_tensor("ro", [P, KH, C], bf16, kind="ExternalOutput")
    sp = nc.dram_tensor("sp", [P, NCH, KH, NT], bf16, kind="ExternalOutput")

    runs = _pack_runs(C)

    with tile.TileContext(nc) as tc:
        from contextlib import ExitStack

        with ExitStack() as ctx:
            const = ctx.enter_context(tc.tile_pool(name="const", bufs=1))
            xpool = ctx.enter_context(tc.tile_pool(name="xpool", bufs=2))
            wpool = ctx.enter_context(tc.tile_pool(name="wpool", bufs=10))
            wdpool = ctx.enter_context(tc.tile_pool(name="wdpool", bufs=8))
            hsp = ctx.enter_context(tc.tile_pool(name="hsp", bufs=2))
            hbuf = ctx.enter_context(tc.tile_pool(name="hbuf", bufs=2))
            outp = ctx.enter_context(tc.tile_pool(name="outp", bufs=2))
            psum = ctx.enter_context(tc.tile_pool(name="psum", bufs=8, space="PSUM"))
            hTp = ctx.enter_context(tc.tile_pool(name="hTp", bufs=1))

            # ring alternator for input streams: both HWDGE FIFOs carry every
            # stream in consumption order, halving the skew a dedicated-ring
            # split would produce under 50/50 packet arbitration.
            rings = [nc.sync, nc.scalar]
            rstate = [0]

            def ring():
                r = rings[rstate[0] & 1]
                rstate[0] += 1
                return r

            # --- PE warm-up from a memset tile: no DMA dependency, so the
            # clock ramp (~3.4 us at boot) overlaps the DMA cold start, which
            # delivers only ~0.15 MB/us until ~26 us.
            wz = const.tile([P, 512], bf16, name="wz")
            nc.gpsimd.memset(wz, 0.0)
            warm = psum.tile([P, 512], f32, tag="ps", name="warm")
            for _ in range(12):
                nc.tensor.matmul(warm, wz[:, 0:P], wz[:, :],
                                 start=True, stop=True)

            # --- input loads, subtiled <=0.5 MB, in consumption order ---
            xeT = const.tile([P, KH, C], bf16)

            xts = [None] * NCH

            def load_x(t, nq=2):
                # k-quarters/halves: chunk matmuls unblock per k-slice
                xts[t] = xpool.tile([P, KH, NT], bf16, tag="xt", name=f"xt{t}")
                kq = KH // nq
                for hh in range(nq):
                    sl = slice(hh * kq, (hh + 1) * kq)
                    ring().dma_start(out=xts[t][:, sl, :],
                                     in_=x3.ap()[:, t, sl, :])

            wsg_sb = const.tile([P, KH, ISH], bf16)
            wsu_sb = const.tile([P, KH, ISH], bf16)
            wsd_sb = const.tile([P, KSH, H], bf16)

            def load_shared_w(wsb, src, nq=4):
                nk = wsb.shape[1]
                kq = nk // nq
                for hh in range(nq):
                    sl = slice(hh * kq, (hh + 1) * kq)
                    ring().dma_start(out=wsb[:, sl, :], in_=src.ap()[:, sl, :])

            hT = hTp.tile([P, MI, C], bf16)
            ro_sb = const.tile([P, KH, C], bf16, name="ro_sb")

            KHH = KH // 2  # half-group k tiles

            def routed_gu_group(g):
                """Routed expert gate/up for one 512-wide I group -> hT.

                fp8 weights stream in 0.5 MB half-tiles. Group 0 (the cold-DMA
                window) gets 0.25 MB gate quarters hand-placed on the rings,
                interleaved with the xeT halves, so both FIFOs deliver the
                first matmuls' operands concurrently."""
                nq = 4 if g == 0 else 2
                qt = {}

                def wtile(w, hh, kq):
                    wB = wpool.tile([P, kq, 512], f8, tag="w",
                                    name=f"w{g}_{w}_{hh}",
                                    padded_shape=[P, KHH, 512])
                    qt[(w, hh)] = (wB, kq)
                    return wB, wgu.ap()[:, g, w, hh * kq:(hh + 1) * kq, :]

                if g == 0:
                    KHQ = KH // 4
                    gq = [wtile(0, hh, KHQ) for hh in range(4)]
                    uh = [wtile(1, hh, KHH) for hh in range(2)]
                    xe_sl = [slice(0, KHH), slice(KHH, KH)]
                    nc.sync.dma_start(out=xeT[:, xe_sl[0], :],
                                      in_=xe3.ap()[:, xe_sl[0], :])
                    nc.scalar.dma_start(out=gq[0][0], in_=gq[0][1])
                    nc.sync.dma_start(out=gq[1][0], in_=gq[1][1])
                    nc.scalar.dma_start(out=xeT[:, xe_sl[1], :],
                                        in_=xe3.ap()[:, xe_sl[1], :])
                    nc.scalar.dma_start(out=gq[2][0], in_=gq[2][1])
                    nc.sync.dma_start(out=gq[3][0], in_=gq[3][1])
                    nc.sync.dma_start(out=uh[0][0], in_=uh[0][1])
                    nc.scalar.dma_start(out=uh[1][0], in_=uh[1][1])
                else:
                    for w in range(2):
                        for hh in range(2):
                            wB, src = wtile(w, hh, KHH)
                            ring().dma_start(out=wB, in_=src)
                gps, ups, mmap = [], [], []
                for ri, q in enumerate(runs):
                    gps.append(psum.tile([P, q, C], f32, tag="ps",
                                         name=f"gps{g}_{ri}"))
                    ups.append(psum.tile([P, q, C], f32, tag="ps",
                                         name=f"ups{g}_{ri}"))
                    for j in range(q):
                        mmap.append((ri, j))
                # group 0 (cold-DMA window) iterates mi-inner so each arriving
                # weight subtile unblocks work for all m-tiles; later groups
                # iterate k-inner, which keeps the PSUM destination fixed for
                # 16 consecutive matmuls (saves the per-instruction dst-switch
                # bubble) since their weights are prefetched anyway.
                for w, ps_tiles in ((0, gps), (1, ups)):
                    nw = nq if w == 0 else 2
                    kq = KH // nw
                    if g == 0:
                        order = [(k, mi) for k in range(KH) for mi in range(4)]
                    else:
                        order = [(k, mi) for mi in range(4) for k in range(KH)]
                    for k, mi in order:
                        wB, _ = qt[(w, k // kq)]
                        ri, j = mmap[mi]
                        # start only on the first write to each PSUM bank:
                        # start=True clears the whole bank's has_written.
                        st = dict(start=(k == 0 and j == 0),
                                  stop=(k == KH - 1))
                        nc.tensor.matmul(ps_tiles[ri][:, j, :],
                                         wB[:, k % kq, mi * P:(mi + 1) * P],
                                         xeT[:, k, :], **st)
                off = 0
                for ri, q in enumerate(runs):
                    h_sb = hbuf.tile([P, q, C], bf16, tag="hrb",
                                     name=f"hrb{g}_{ri}")
                    # dequant: PSUM holds 128*gate; silu(in/128)
                    nc.scalar.activation(out=h_sb, in_=gps[ri], func=SILU,
                                         scale=1.0 / WS)
                    # h = silu(gate) * (128*up): hT carries a 128x scale
                    nc.vector.tensor_tensor(hT[:, g * 4 + off:g * 4 + off + q, :],
                                            h_sb, ups[ri], MULT)
                    off += q

            hss = [None] * NCH

            def shared_gu(t):
                """Shared expert gate/up for one 256-token chunk -> hs[t]."""
                xt = xts[t]
                # pack two [P, NT] fp32 accumulators per PSUM bank
                sg = [psum.tile([P, 2, NT], f32, tag="ps", name=f"sg{t}_{r}")
                      for r in range(2)]
                su = [psum.tile([P, 2, NT], f32, tag="ps", name=f"su{t}_{r}")
                      for r in range(2)]
                # k-inner: fixed PSUM destination for 16 consecutive matmuls
                for ps_tiles, wB in ((sg, wsg_sb), (su, wsu_sb)):
                    for m in range(KSH):
                        for k in range(KH):
                            st = dict(start=(k == 0 and m % 2 == 0),
                                      stop=(k == KH - 1))
                            nc.tensor.matmul(ps_tiles[m // 2][:, m % 2, :],
                                             wB[:, k, m * P:(m + 1) * P],
                                             xt[:, k, :], **st)
                hs = hsp.tile([P, KSH, NT], bf16, tag="hs", name=f"hs{t}")
                hss[t] = hs
                for r in range(2):
                    htmp = hbuf.tile([P, 2, NT], bf16, tag="hsb",
                                     name=f"htmp{t}_{r}")
                    nc.scalar.activation(out=htmp, in_=sg[r], func=SILU)
                    nc.vector.tensor_tensor(hs[:, 2 * r:2 * r + 2, :], htmp,
                                            su[r], MULT)

            def shared_down(t, final=False):
                """Shared expert down-proj for chunk t -> sp.

                The final chunk runs last in the kernel: its sp output leaves
                in 0.25 MB subtiles (2 KB DRAM lines) as each 4-m2 block's
                copies land, so the kernel tail is one short transfer."""
                hs = hss[t]
                sp_sb = outp.tile([P, KH, NT], bf16, tag="spsb", name=f"spsb{t}",
                                  bufs=1)
                for m2 in range(KH):
                    ps = psum.tile([P, NT], f32, tag="ps", name=f"sps{t}_{m2}")
                    for k2 in range(KSH):
                        nc.tensor.matmul(ps, wsd_sb[:, k2, m2 * P:(m2 + 1) * P],
                                         hs[:, k2, :],
                                         start=(k2 == 0), stop=(k2 == KSH - 1))
                    nc.vector.tensor_copy(out=sp_sb[:, m2, :], in_=ps)
                    if final and m2 % 4 == 3:
                        sl = slice(m2 - 3, m2 + 1)
                        nc.scalar.dma_start(out=sp.ap()[:, t, sl, :],
                                            in_=sp_sb[:, sl, :])
                if not final:
                    nc.gpsimd.dma_start(out=sp.ap()[:, t], in_=sp_sb)

            # wd streams as 16 x 0.5 MB fp8 chunks (8 k2-tiles each); the
            # first six are hoisted into the gate/up phase, the rest queue at
            # the start of the down phase and fire as ring slots free.
            wd_tiles = {}

            def prefetch_wd(c):
                if c in wd_tiles or c >= 16:
                    return
                wdB = wdpool.tile([P, KHH, 512], f8, tag="wd",
                                  name=f"wdB{c}")
                g2, q = c // 4, c % 4
                ring().dma_start(
                    out=wdB, in_=wd3.ap()[:, g2, q * KHH:(q + 1) * KHH, :])
                wd_tiles[c] = wdB

            def routed_down_group(g2):
                """Routed expert down-proj for one 512-wide H group -> ro_sb."""
                for c in range(6, 16):
                    prefetch_wd(c)
                dps, mmap = [], []
                for ri, q in enumerate(runs):
                    dps.append(psum.tile([P, q, C], f32, tag="ps",
                                         name=f"dps{g2}_{ri}"))
                    for j in range(q):
                        mmap.append((ri, j))
                # k2-inner: fixed PSUM destination for 32 consecutive matmuls
                for mi in range(4):
                    ri, j = mmap[mi]
                    for k2 in range(MI):
                        wb = wd_tiles[g2 * 4 + k2 // KHH]
                        kk = k2 % KHH
                        st = dict(start=(k2 == 0 and j == 0),
                                  stop=(k2 == MI - 1))
                        nc.tensor.matmul(dps[ri][:, j, :],
                                         wb[:, kk, mi * P:(mi + 1) * P],
                                         hT[:, k2, :], **st)
                # ro carries a 128^2 scale (fp8 wd x fp8-gated h); host
                # descales by 2^-14 exactly.
                off = 0
                for ri, q in enumerate(runs):
                    nc.vector.tensor_copy(out=ro_sb[:, g2 * 4 + off:
                                                    g2 * 4 + off + q, :],
                                          in_=dps[ri])
                    off += q
                # ro leaves in 8-m-tile halves: 2304 B DRAM lines (one
                # 288 B line per m-tile would run far below line rate)
                if g2 == 1:
                    nc.gpsimd.dma_start(out=ro.ap()[:, 0:8, :],
                                        in_=ro_sb[:, 0:8, :])
                elif g2 == 3:
                    nc.sync.dma_start(out=ro.ap()[:, 8:16, :],
                                      in_=ro_sb[:, 8:16, :])

            # Issue order == consumption order (the ring alternator assumes
            # it). Shared units are placed so their input-byte needs fit the
            # HBM rate alongside the routed weight stream.
            routed_gu_group(0)
            load_x(0, nq=4)
            load_shared_w(wsg_sb, wsg3, nq=4)
            load_shared_w(wsu_sb, wsu3, nq=4)
            shared_gu(0)
            routed_gu_group(1)
            load_x(1)
            shared_gu(1)
            routed_gu_group(2)
            load_shared_w(wsd_sb, wsd3, nq=2)
            shared_down(0)
            routed_gu_group(3)
            load_x(2)
            shared_gu(2)
            routed_gu_group(4)
            load_x(3)
            shared_gu(3)
            routed_gu_group(5)
            shared_down(1)
            for c in range(4):
                prefetch_wd(c)
            routed_gu_group(6)
            for c in range(4, 6):
                prefetch_wd(c)
            routed_gu_group(7)
            shared_down(2)
            routed_down_group(0)
            routed_down_group(1)
            routed_down_group(2)
            routed_down_group(3)
            # the shared chunk-3 down-proj closes the kernel: resident
            # weights, fat-line subtile outputs -> shortest possible tail
            shared_down(3, final=True)

    # Split surplus semaphore waits onto InstEventSemaphore carriers (walrus
    # has a 1-wait limit per instruction).
    import bass_rust
    bass_rust.generate_event_semaphores(nc)
    return nc


def _get_bass(C):
    if C not in _BASS_CACHE:
        _BASS_CACHE[C] = _build_bass(C)
    return _BASS_CACHE[C]


def _q8(w):
    """fp32 weight block -> e3m4 scaled by WS, clipped to the format max."""
    return np.clip(w * WS, -F8MAX, F8MAX).astype(F8)


def kernel(**inputs):
    global LAST_RESULT, LAST_NC
    x = np.ascontiguousarray(np.asarray(inputs["x"], dtype=np.float32))
    w_router = np.asarray(inputs["w_router"], dtype=np.float32)
    ws_gate = np.asarray(inputs["ws_gate"], dtype=np.float32)
    ws_up = np.asarray(inputs["ws_up"], dtype=np.float32)
    ws_down = np.asarray(inputs["ws_down"], dtype=np.float32)
    we_gate = np.asarray(inputs["we_gate"], dtype=np.float32)
    we_up = np.asarray(inputs["we_up"], dtype=np.float32)
    we_down = np.asarray(inputs["we_down"], dtype=np.float32)

    # --- top-1 routing on host (tiny) ---
    logits = x @ w_router                      # [T, E]
    top = np.argmax(logits, axis=1)            # [T]
    tv = logits[np.arange(T), top]
    score = (1.0 / (1.0 + np.exp(-tv))).astype(np.float32)
    all_idxs = [np.nonzero(top == e)[0] for e in range(E)]
    maxn = max(len(i) for i in all_idxs)
    C = min(512, max(P, ((maxn + 15) // 16) * 16))
    # capacity overflow (not hit for balanced routing): process the routed
    # tokens in multiple passes of <= 512; only pass 0's shared output counts.
    n_pass = (maxn + C - 1) // C

    nc = _get_bass(C)
    LAST_NC = nc

    # x3[p, t, k, j] = x[t*NT + j, k*128 + p]
    x3 = x.reshape(NCH, NT, KH, P).transpose(3, 0, 2, 1).astype(BF)

    from concourse.bass_utils import run_bass_kernel_spmd

    out = None
    for p_i in range(n_pass):
        idxs = [i[p_i * C:(p_i + 1) * C] for i in all_idxs]
        in_maps = []
        for e in range(E):
            idx = idxs[e]
            xe = np.zeros((C, H), np.float32)
            if len(idx):
                xe[:len(idx)] = x[idx] * score[idx, None]
            # xe3[p, k, c] = xe[c, k*128 + p]
            xe3 = xe.reshape(C, KH, P).transpose(2, 1, 0).astype(BF)

            # wgu[p, g, w, k, j] = we_{gate,up}[e][k*128 + p, g*512 + j]
            wgu = np.empty((P, 8, 2, KH, 512), F8)
            wgu[:, :, 0] = _q8(we_gate[e]).reshape(KH, P, 8, 512) \
                .transpose(1, 2, 0, 3)
            wgu[:, :, 1] = _q8(we_up[e]).reshape(KH, P, 8, 512) \
                .transpose(1, 2, 0, 3)
            # wd3[p, g2, k2, j] = we_down[e][k2*128 + p, g2*512 + j]
            wd3 = _q8(we_down[e]).reshape(MI, P, 4, 512) \
                .transpose(1, 2, 0, 3)

            # shared-expert shard for this core (bf16)
            wsg3 = ws_gate[:, e * ISH:(e + 1) * ISH].reshape(KH, P, ISH) \
                .transpose(1, 0, 2).astype(BF)
            wsu3 = ws_up[:, e * ISH:(e + 1) * ISH].reshape(KH, P, ISH) \
                .transpose(1, 0, 2).astype(BF)
            wsd3 = ws_down[e * ISH:(e + 1) * ISH].reshape(KSH, P, H) \
                .transpose(1, 0, 2).astype(BF)

            in_maps.append({
                "xe3": xe3, "wgu": wgu, "wd3": wd3, "x3": x3,
                "wsg3": wsg3, "wsu3": wsu3, "wsd3": wsd3,
            })

        res = run_bass_kernel_spmd(nc, in_maps, core_ids=list(range(E)))
        LAST_RESULT = res
        outs = res.results

        if p_i == 0:
            # shared partials: sp[p, t, m2, j] -> [token, h], sum over cores
            spsum = np.zeros((P, NCH, KH, NT), np.float32)
            for e in range(E):
                spsum += outs[e]["sp"].astype(np.float32)
            out = np.ascontiguousarray(
                spsum.transpose(1, 3, 2, 0).reshape(T, H))

        # routed: ro[p, m, c] -> [c, h], scatter back by token index,
        # descaling the fp8 weight scale (WS^2) exactly.
        dsc = np.float32(1.0 / (WS * WS))
        for e in range(E):
            idx = idxs[e]
            if len(idx):
                roe = outs[e]["ro"].astype(np.float32) * dsc
                out[idx] += roe.transpose(2, 1, 0).reshape(C, H)[:len(idx)]
    return out
